# revision 2
# baseline (speedup 1.0000x reference)
"""Trainium2 Bass kernel for one DPMM VB-EM iteration (M-step + E-step).

v2: fp8 pipeline. See kernel.py (v1) for the baseline structure.

Strategy (data-parallel over rows, 8 cores):
  - Each core gets a 187500-row shard, zero-padded to 188416 = 128*1472 rows,
    p-major: row n -> (partition p = n // 1472, chunk i = n % 1472).
  - Features per chunk (row-block of 128): x (4), symmetric quads x_d*x_e
    d<=e (10) = 14, plus a shared ones row. Groups of 8 chunks pack into
    112 feature columns + ones = 113 <= 128.
  - NEFF A (stats): phi streamed in fp8(e4m3); F6A fp8 resident;
    per group one matmul  stats += F6A_g^T @ phi_g  ([128,128] f32 PSUM,
    lhsT = 128-col stationary so fp8 FWL hides the weight load).
  - Host: sums 8 partial stats, computes M-step + E-step coefficients in
    f64, centers per cluster, scales by 256, casts to fp8.
  - NEFF B (E-step): F6B fp8 built + PE-transposed to FT (feature-major)
    ONCE in setup, resident in SBUF. Body: one DoubleRow fp8 matmul per
    4-group super with W as the stationary -> logitsT [128 (c,t), 512
    (g,p)] f32 PSUM -> evac to fp8 (DVE/ACT/GPSIMD round-robin) -> DMA.
    Output is the scaled logit in TRANSPOSED layout [128, NG*128] fp8;
    host un-permutes, /256, exp, row-normalizes (softmax).

Self-contained: hardcodes shapes for N=1500000, D=4, T=16, 8 cores.
"""
import os
import sys

os.environ.setdefault("CONCOURSE_KEEP_NRT", "1")
sys.path.insert(0, "/opt/trn_rl_repo")

from contextlib import ExitStack

import ml_dtypes
import numpy as np

import concourse.bass as bass
import concourse.tile as tile
from concourse import bacc
from concourse import mybir
from concourse.bass_utils import run_bass_kernel_spmd

F32 = mybir.dt.float32
BF16 = mybir.dt.bfloat16
F8 = mybir.dt.float8e4
NP_BF16 = ml_dtypes.bfloat16
NP_F8 = ml_dtypes.float8_e4m3

# ---------------- problem geometry ----------------
N_TOTAL = 1_500_000
D = 4
T = 16
NCORES = 8
RSH = N_TOTAL // NCORES          # rows per core (187500)
P = 128                          # partitions
CPG = 8                          # chunks per feature group
NG = 184                         # groups per core
M = CPG * NG                     # chunks per core (1472)
RPAD = P * M                     # padded rows per core (188416)
SEG = 57                         # transpose segment: 4 chunks * 14 + ones
NSUP = NG // 4                   # E-step supers (4 groups each) = 46
SCALE = 256.0                    # W (and logit) scaling for fp8 range

ALPHA_DP = 1e-3
LOG2 = float(np.log(2.0))

# symmetric quad feature order
PAIRS = [(d, e) for d in range(D) for e in range(d, D)]

A_TILES = 4                      # phi stream tiles (46 groups each)
B_DMA_SUPERS = 4                 # supers per output DMA batch


def _build_f6a(nc, f6, xv):
    """F6A fp8 [P, NG*128] from x [P, M, 4] bf16 (stats layout).

    Group block cols: c*14+j for c in 0..8, j in 0..14 (x then quads);
    ones at col 112; cols 113..127 zero.
    """
    f6v = f6[:].rearrange("p (g f) -> p g f", f=128)
    nc.vector.memset(f6v[:, :, 112:113], 1.0)
    nc.vector.memset(f6v[:, :, 113:128], 0.0)
    x4 = xv.rearrange("p (g c) d -> p g c d", c=CPG)
    for c in range(CPG):
        eng = nc.vector if c % 2 == 0 else nc.gpsimd
        eng.tensor_copy(f6v[:, :, c * 14:c * 14 + 4], x4[:, :, c, :])
    for q, (d, e) in enumerate(PAIRS):
        dst = f6v[:, :, 4 + q:4 + q + 14 * CPG:14]        # [P, NG, 8]
        in0 = x4[:, :, :, d]
        in1 = x4[:, :, :, e]
        eng = nc.vector if q % 2 == 0 else nc.gpsimd
        eng.tensor_mul(dst, in0, in1)


def build_stats_nc(num_devices=NCORES, repeat=1, loop=None, dma_queues=1,
                   stages=2):
    nc = bacc.Bacc("TRN2", target_bir_lowering=False, debug=False,
                   num_devices=num_devices)
    x = nc.dram_tensor("x", [RPAD, D], BF16, kind="ExternalInput")
    phi = nc.dram_tensor("phi", [RPAD, T], F8, kind="ExternalInput")
    stats = nc.dram_tensor("stats", [P, P], F32, kind="ExternalOutput")

    xr = x.ap().rearrange("(p i) d -> p i d", p=P)
    phir = phi.ap().rearrange("(p i) t -> p i t", p=P)

    with tile.TileContext(nc) as tc, ExitStack() as ctx:
        xpool = ctx.enter_context(tc.tile_pool(name="xp", bufs=1))
        f6pool = ctx.enter_context(tc.tile_pool(name="f6p", bufs=1))
        phipool = ctx.enter_context(tc.tile_pool(name="php", bufs=3))
        pspool = ctx.enter_context(
            tc.tile_pool(name="psp", bufs=1, space=bass.MemorySpace.PSUM))
        opool = ctx.enter_context(tc.tile_pool(name="op", bufs=1))

        x_sb = xpool.tile([P, M * D], BF16)
        xv = x_sb[:].rearrange("p (i d) -> p i d", d=D)
        nc.sync.dma_start(out=xv, in_=xr)

        f6 = f6pool.tile([P, NG * 128], F8)
        _build_f6a(nc, f6, xv)

        gpt = NG // A_TILES                       # groups per phi tile (46)
        cpt = gpt * CPG                           # chunks per phi tile (368)
        if stages >= 2:
            ps = pspool.tile([P, P], F32)

        def a_body():
            for ti in range(A_TILES):
                pt = phipool.tile([P, cpt * T], F8, tag="pt")
                ptr = pt[:].rearrange("p (i t) -> p i t", t=T)
                if dma_queues == 1:
                    nc.sync.dma_start(
                        out=ptr, in_=phir[:, ti * cpt:(ti + 1) * cpt, :])
                else:
                    h = cpt // 2
                    nc.sync.dma_start(
                        out=ptr[:, 0:h, :],
                        in_=phir[:, ti * cpt:ti * cpt + h, :])
                    nc.scalar.dma_start(
                        out=ptr[:, h:cpt, :],
                        in_=phir[:, ti * cpt + h:(ti + 1) * cpt, :])
                ptv = pt[:].rearrange("p (gl f) -> p gl f", f=CPG * T)
                if stages < 2:
                    continue
                for gl in range(gpt):
                    g = ti * gpt + gl
                    nc.tensor.matmul(
                        ps[:],
                        lhsT=f6[:, g * 128:(g + 1) * 128],
                        rhs=ptv[:, gl, :],
                        start=(g == 0), stop=(g == NG - 1))

        if loop is not None:
            with tc.For_i(0, loop):
                for _rep in range(repeat):
                    a_body()
        else:
            for _rep in range(repeat):
                a_body()

        st_sb = opool.tile([P, P], F32)
        if stages >= 2:
            nc.scalar.copy(st_sb[:], ps[:])
        else:
            nc.scalar.copy(st_sb[:], f6[:, 0:P * 4].bitcast(F32))
        nc.sync.dma_start(out=stats.ap(), in_=st_sb[:])
    nc.compile()
    return nc


def _build_f6b(nc, f6, xv):
    """F6B bf16 [P, NG*114] from x (E-step transpose layout; bf16 because
    the fp8 PE-transpose path needs stride-2 output — we convert to fp8 in
    the PSUM->FT evacuation copy instead).

    Group block: two 57-col segments (halves); segment i cols:
    i*57 + cl*14 + j for cl in 0..4 (chunk c = 4i+cl), ones at i*57+56.
    """
    f6v = f6[:].rearrange("p (g f) -> p g f", f=2 * SEG)
    nc.vector.memset(f6v[:, :, 56:57], 1.0)
    nc.vector.memset(f6v[:, :, 113:114], 1.0)
    x4 = xv.rearrange("p (g c) d -> p g c d", c=CPG)
    for c in range(CPG):
        i, cl = c // 4, c % 4
        base = i * SEG + cl * 14
        eng = nc.vector if c % 2 == 0 else nc.gpsimd
        eng.tensor_copy(f6v[:, :, base:base + 4], x4[:, :, c, :])
    for q, (d, e) in enumerate(PAIRS):
        for i in range(2):
            base = i * SEG + 4 + q
            dst = f6v[:, :, base:base + 14 * 3 + 1:14]
            in0 = x4[:, :, 4 * i:4 * i + 4, d]
            in1 = x4[:, :, 4 * i:4 * i + 4, e]
            eng = nc.vector if (2 * q + i) % 2 == 0 else nc.gpsimd
            eng.tensor_mul(dst, in0, in1)


def build_estep_nc(num_devices=NCORES, repeat=1, loop=None, f32_supers=0,
                   out_queues=2, paired_evac=True, dr=False, stages=3):
    nc = bacc.Bacc("TRN2", target_bir_lowering=False, debug=False,
                   num_devices=num_devices)
    x = nc.dram_tensor("x", [RPAD, D], BF16, kind="ExternalInput")
    wshape = [SEG, 2 * P] if dr else [2 * SEG, P]
    w = nc.dram_tensor("w", wshape, F8, kind="ExternalInput")
    ident = nc.dram_tensor("ident", [P, P], BF16, kind="ExternalInput")
    lg = nc.dram_tensor("lg", [P, NG * P], F8, kind="ExternalOutput")
    if f32_supers:
        lgf = nc.dram_tensor("lgf", [P, f32_supers * 512], F32,
                             kind="ExternalOutput")

    xr = x.ap().rearrange("(p i) d -> p i d", p=P)

    with tile.TileContext(nc) as tc, ExitStack() as ctx:
        xpool = ctx.enter_context(tc.tile_pool(name="xp", bufs=1))
        f6pool = ctx.enter_context(tc.tile_pool(name="f6p", bufs=1))
        cpool = ctx.enter_context(tc.tile_pool(name="cp", bufs=1))
        ftpool = ctx.enter_context(tc.tile_pool(name="ftp", bufs=1))
        tppool = ctx.enter_context(
            tc.tile_pool(name="tpp", bufs=2, space=bass.MemorySpace.PSUM))
        lpspool = ctx.enter_context(
            tc.tile_pool(name="lps", bufs=3, space=bass.MemorySpace.PSUM))
        epool = ctx.enter_context(tc.tile_pool(name="ep", bufs=3))

        x_sb = xpool.tile([P, M * D], BF16)
        xv = x_sb[:].rearrange("p (i d) -> p i d", d=D)
        nc.sync.dma_start(out=xv, in_=xr)

        w_sb = cpool.tile(wshape, F8, tag="w")
        nc.sync.dma_start(out=w_sb[:], in_=w.ap())
        id_sb = cpool.tile([P, P], BF16, tag="id")
        nc.sync.dma_start(out=id_sb[:], in_=ident.ap())

        f6 = f6pool.tile([P, NG * 2 * SEG], BF16)
        _build_f6b(nc, f6, xv)

        # FT: transposed features, resident.
        # dr: [SEG parts, per super (i, gl, p) = 1024]; the two 57-row
        # halves land on the same partitions at different free offsets.
        # plain: [2*SEG parts, per super (gl, p) = 512]; whole 114-col
        # group segment transposed at once.
        if dr:
            ft = ftpool.tile([SEG, NSUP * 1024], F8)
            for s in range(NSUP):
                tp = tppool.tile([SEG, 1024], BF16, tag="tp")
                for i in range(2):
                    for gl in range(4):
                        g = s * 4 + gl
                        nc.tensor.matmul(
                            tp[:, (i * 4 + gl) * P:(i * 4 + gl + 1) * P],
                            lhsT=f6[:, g * 2 * SEG + i * SEG:
                                    g * 2 * SEG + (i + 1) * SEG],
                            rhs=id_sb[:], is_transpose=True,
                            start=True, stop=True)
                if s % 2 == 1:
                    nc.scalar.copy(ft[:, s * 1024:(s + 1) * 1024], tp[:])
                else:
                    nc.vector.tensor_copy(ft[:, s * 1024:(s + 1) * 1024],
                                          tp[:])
        else:
            ft = ftpool.tile([2 * SEG, NSUP * 512], F8)
            for s in range(NSUP):
                tp = tppool.tile([2 * SEG, 512], BF16, tag="tp")
                for gl in range(4):
                    g = s * 4 + gl
                    nc.tensor.matmul(
                        tp[:, gl * P:(gl + 1) * P],
                        lhsT=f6[:, g * 2 * SEG:(g + 1) * 2 * SEG],
                        rhs=id_sb[:], is_transpose=True,
                        start=True, stop=True)
                if s % 2 == 1:
                    nc.scalar.copy(ft[:, s * 512:(s + 1) * 512], tp[:])
                else:
                    nc.vector.tensor_copy(ft[:, s * 512:(s + 1) * 512],
                                          tp[:])

        wv = w_sb[:].rearrange("k (i m) -> k i m", i=2) if dr else w_sb[:]

        def super_mm(out_ap, s):
            if dr:
                nc.tensor.matmul(
                    out_ap, lhsT=wv,
                    rhs=ft[:, s * 1024:(s + 1) * 1024].rearrange(
                        "k (i n) -> k i n", i=2),
                    perf_mode=mybir.MatmulPerfMode.DoubleRow,
                    start=True, stop=True)
            else:
                nc.tensor.matmul(
                    out_ap, lhsT=wv,
                    rhs=ft[:, s * 512:(s + 1) * 512],
                    start=True, stop=True)

        # DVE:ACT evac split weighted by clock (0.96 vs 1.2 GHz)
        npair = NSUP // 2
        act_share = [
            (sp * 13) // npair > ((sp - 1) * 13) // npair
            for sp in range(npair)]

        def b_body():
            if not paired_evac:
                for s in range(NSUP):
                    l_ps = lpspool.tile([P, 512], F32, tag="lps")
                    super_mm(l_ps[:], s)
                    if s % B_DMA_SUPERS == 0:
                        e_t = epool.tile([P, B_DMA_SUPERS * 512], F8, tag="e")
                        dma_s0 = s
                    dst = e_t[:, (s % B_DMA_SUPERS) * 512:
                              (s % B_DMA_SUPERS + 1) * 512]
                    if s % 2 == 1:
                        nc.scalar.copy(dst, l_ps[:])
                    else:
                        nc.vector.tensor_copy(dst, l_ps[:])
                    if s % B_DMA_SUPERS == B_DMA_SUPERS - 1 or s == NSUP - 1:
                        filled = (s - dma_s0 + 1) * 512
                        nc.sync.dma_start(
                            out=lg.ap()[:, dma_s0 * 512:dma_s0 * 512 + filled],
                            in_=e_t[:, 0:filled])
                return
            for sp in range(npair):
                l_ps = lpspool.tile([P, 1024], F32, tag="lps")
                for h in range(2):
                    s = sp * 2 + h
                    super_mm(l_ps[:, h * 512:(h + 1) * 512], s)
                if stages < 2:
                    if sp == npair - 1:
                        e_t = epool.tile([P, 2048], F8, tag="e")
                        nc.vector.tensor_copy(e_t[:, 0:1024], l_ps[:])
                        nc.sync.dma_start(out=lg.ap()[:, 0:2048], in_=e_t[:])
                    continue
                if sp % 2 == 0:
                    e_t = epool.tile([P, 2048], F8, tag="e")
                    dma_p0 = sp
                dst = e_t[:, (sp % 2) * 1024:(sp % 2 + 1) * 1024]
                if act_share[sp]:
                    nc.scalar.copy(dst, l_ps[:])
                else:
                    nc.vector.tensor_copy(dst, l_ps[:])
                if sp % 2 == 1 and stages >= 3:
                    deng = (nc.scalar if (out_queues > 1 and (sp // 2) % 2)
                            else nc.sync)
                    deng.dma_start(
                        out=lg.ap()[:, dma_p0 * 1024:(dma_p0 + 2) * 1024],
                        in_=e_t[:])

        if loop is not None:
            with tc.For_i(0, loop):
                for _rep in range(repeat):
                    b_body()
        else:
            for _rep in range(repeat):
                b_body()
    nc.compile()
    return nc


# ---------------- host middle step ----------------

def _digamma(xx):
    xx = np.asarray(xx, dtype=np.float64)
    acc = np.zeros_like(xx)
    for k in range(8):
        acc += 1.0 / (xx + k)
    y = xx + 8.0
    y2 = 1.0 / (y * y)
    ser = np.log(y) - 0.5 / y - y2 * (1.0 / 12.0 - y2 * (1.0 / 120.0 - y2 / 252.0))
    return ser - acc


def _compute_W(stats_sum, priorMu, priorKappa, priorPsi, priorNu, dr=False):
    """stats_sum [128,128] f64 -> W f64 (scaled, centered).

    dr=True: [57, 256] DoubleRow layout; else [114, 128] plain layout."""
    Nk = np.zeros(T)
    Sx = np.zeros((D, T))
    Sxx = np.zeros((D, D, T))
    for c in range(CPG):
        blk = stats_sum[c * 14:c * 14 + 14, c * 16:(c + 1) * 16]
        Sx += blk[0:4, :]
        for q, (d, e) in enumerate(PAIRS):
            Sxx[d, e] += blk[4 + q]
            if d != e:
                Sxx[e, d] += blk[4 + q]
        Nk += stats_sum[112, c * 16:(c + 1) * 16]

    mu0 = np.asarray(priorMu, np.float64).reshape(D, 1)
    k0 = float(np.asarray(priorKappa).reshape(-1)[0])
    Psi0 = np.asarray(priorPsi, np.float64)
    nu0 = float(np.asarray(priorNu).reshape(-1)[0])

    g1 = 1.0 + Nk
    tail = np.cumsum(Nk[::-1])[::-1]
    g2 = ALPHA_DP + (tail - Nk)

    prior11 = Psi0 + k0 * (mu0 @ mu0.T)
    S = np.transpose(Sxx, (2, 0, 1))
    T12 = k0 * mu0 + Sx
    kappa = k0 + Nk
    mu = T12 / kappa[None, :]
    nu = Nk + nu0
    Psi = prior11[None] + S - kappa[:, None, None] * np.einsum(
        'dt,et->tde', mu, mu)

    dg_sum = _digamma(g1 + g2)
    dg1 = _digamma(g1) - dg_sum
    dg2 = _digamma(g2) - dg_sum
    term2 = np.cumsum(dg2) - dg2

    Psi_inv = np.linalg.inv(Psi)
    sign, logdet = np.linalg.slogdet(Psi)
    Lam = nu[:, None, None] * Psi_inv
    eta2 = np.einsum('tde,et->td', Lam, mu)
    eta3 = -_digamma(0.5 * nu) - D * LOG2 + logdet
    quad = np.einsum('dt,tde,et->t', mu, Psi_inv, mu)
    eta4 = -0.5 * D / kappa - 0.5 * nu * quad

    const = dg1 + term2 - 0.5 * eta3 + eta4
    A = -0.5 * Lam

    # C [15, T]: linear (4), sym quads (10, off-diag doubled), const
    C = np.zeros((15, T))
    C[0:4] = eta2.T
    for q, (d, e) in enumerate(PAIRS):
        C[4 + q] = A[:, d, e] * (1.0 if d == e else 2.0)
    C[14] = const
    # center across clusters (softmax-invariant), scale into fp8 range
    C = (C - C.mean(axis=1, keepdims=True)) * SCALE

    if dr:
        W = np.zeros((SEG, 2, P))
        for c in range(CPG):
            i, cl = c // 4, c % 4
            W[cl * 14:cl * 14 + 14, i, c * 16:(c + 1) * 16] = C[0:14]
        W[56, 0, :] = np.tile(C[14] / 2.0, CPG)
        W[56, 1, :] = np.tile(C[14] / 2.0, CPG)
        return W.reshape(SEG, 2 * P)
    W = np.zeros((2 * SEG, P))
    for c in range(CPG):
        i, cl = c // 4, c % 4
        k0r = i * SEG + cl * 14
        W[k0r:k0r + 14, c * 16:(c + 1) * 16] = C[0:14]
    # the two ones rows (56, 113) each carry half the constant
    W[56, :] = np.tile(C[14] / 2.0, CPG)
    W[113, :] = np.tile(C[14] / 2.0, CPG)
    return W


# ---------------- top-level kernel ----------------

_CACHE = {}


def _get_ncs():
    if "stats" not in _CACHE:
        _CACHE["stats"] = build_stats_nc()
        _CACHE["estep"] = build_estep_nc()
    return _CACHE["stats"], _CACHE["estep"]


def kernel(data, Phi, priorMu, priorKappa, priorPsi, priorNu):
    data = np.asarray(data)
    Phi = np.asarray(Phi)
    nc_stats, nc_estep = _get_ncs()

    xs, ps = [], []
    for c in range(NCORES):
        xc = np.zeros((RPAD, D), NP_BF16)
        pc = np.zeros((RPAD, T), NP_F8)
        xc[:RSH] = data[c * RSH:(c + 1) * RSH].astype(NP_BF16)
        pc[:RSH] = Phi[c * RSH:(c + 1) * RSH].astype(NP_F8)
        xs.append(xc)
        ps.append(pc)

    in_maps = [{"x": xs[c], "phi": ps[c]} for c in range(NCORES)]
    res_a = run_bass_kernel_spmd(nc_stats, in_maps, core_ids=list(range(NCORES)))
    stats_sum = np.zeros((P, P), np.float64)
    for r in res_a.results:
        stats_sum += np.asarray(r["stats"], np.float64)

    W = _compute_W(stats_sum, priorMu, priorKappa, priorPsi, priorNu)
    Wb = np.ascontiguousarray(W.astype(NP_F8))
    ident = np.ascontiguousarray(np.eye(P).astype(NP_BF16))

    in_maps_b = [{"x": xs[c], "w": Wb, "ident": ident} for c in range(NCORES)]
    res_b = run_bass_kernel_spmd(nc_estep, in_maps_b, core_ids=list(range(NCORES)))

    out = np.empty((N_TOTAL, T), np.float32)
    for c in range(NCORES):
        lgc = np.asarray(res_b.results[c]["lg"]).astype(np.float32)
        # [m=(cc,t), n=(g,p)] -> rows (p, g, cc)
        lgc = lgc.reshape(CPG, T, NG, P).transpose(3, 2, 0, 1).reshape(RPAD, T)
        lgc = lgc[:RSH] * (1.0 / SCALE)
        np.exp(lgc, out=lgc)
        lgc /= lgc.sum(axis=1, keepdims=True)
        out[c * RSH:(c + 1) * RSH] = lgc
    return out
